# revision 9
# baseline (speedup 1.0000x reference)
"""Trainium2 Bass kernel for per-head-projection MHA + residual + LayerNorm.

Problem shapes (hardcoded): B=4, S=2048, E=512, H=8, DK=64, fp32.

Sharding: 8 cores, core c -> (batch b = c//2, query-half qh = c%2).
Each core computes the full transformer block for its 1024 query rows
(using the full 2048-row K/V of its batch), so per-core outputs are
disjoint slices of the final [4, 2048, 512] output and no collectives
are needed.

Device-side layout strategy: activations are kept transposed
([feature, seq]) so that every matmul contracts on the partition dim
with zero on-chip transposes in the attention hot loop:
  - qT/kT: [dk, seq] per head (stored as head-pairs on 128 partitions)
  - scoresT: [keys, queries] = kT_tile.T @ qT
  - exp via ScalarE with the 1/sqrt(dk) scale folded in
  - PV: ctxT[dk, q] = v_aug[t, dk+1].T @ exp[t, q]; the extra ones
    column of v_aug yields the softmax denominator for free (row 64)
  - final linear consumes ctxT slices as the stationary operand
"""

import sys

sys.path.insert(0, "/opt/trn_rl_repo")

import numpy as np

B, S, E, H, DK = 4, 2048, 512, 8, 64
NCORES = 8
SQ = (B * S) // NCORES  # 1024 query rows per core
HD = H * DK  # 512
PAIRS = H // 2  # head pairs stacked on 128 partitions
LN_EPS = 1e-5

_PROGRAM_CACHE = {}


def _build_program():
    import concourse.bass as bass
    import concourse.mybir as mybir
    import concourse.tile as tile
    from concourse import bacc
    from concourse.masks import make_identity

    dt = mybir.dt
    f32 = dt.float32
    f32r = dt.float32r
    bf16 = dt.bfloat16
    AF = mybir.ActivationFunctionType

    def r(ap):
        return ap.bitcast(f32r)

    nc = bacc.Bacc("TRN2", target_bir_lowering=False, debug=False)

    # ---- DRAM I/O ----
    Qs_d = nc.dram_tensor("Qs", [SQ, E], f32, kind="ExternalInput").ap()
    Kf_d = nc.dram_tensor("Kf", [S, E], f32, kind="ExternalInput").ap()
    Vf_d = nc.dram_tensor("Vf", [S, E], f32, kind="ExternalInput").ap()
    Wq_d = nc.dram_tensor("Wq", [E, HD], f32r, kind="ExternalInput").ap()
    Wk_d = nc.dram_tensor("Wk", [E, HD], f32r, kind="ExternalInput").ap()
    Wv_d = nc.dram_tensor("Wv", [E, HD], f32r, kind="ExternalInput").ap()
    Wf_d = nc.dram_tensor("Wf", [HD, E], f32r, kind="ExternalInput").ap()
    bq_d = nc.dram_tensor("bq_t", [128, PAIRS], f32, kind="ExternalInput").ap()
    bk_d = nc.dram_tensor("bk_t", [128, PAIRS], f32, kind="ExternalInput").ap()
    bv_d = nc.dram_tensor("bv_t", [DK, H], f32r, kind="ExternalInput").ap()
    bf_d = nc.dram_tensor("bf_r", [1, E], f32, kind="ExternalInput").ap()
    ga_d = nc.dram_tensor("gamma_r", [1, E], f32r, kind="ExternalInput").ap()
    be_d = nc.dram_tensor("beta_r", [1, E], f32r, kind="ExternalInput").ap()
    Out_d = nc.dram_tensor("Out", [SQ, E], f32, kind="ExternalOutput").ap()

    with tile.TileContext(nc) as tc:
        from contextlib import ExitStack

        with ExitStack() as ctx:
            const_p = ctx.enter_context(tc.tile_pool(name="const", bufs=1))
            w_p = ctx.enter_context(tc.tile_pool(name="weights", bufs=1))
            act_p = ctx.enter_context(tc.tile_pool(name="acts", bufs=1))
            xt_p = ctx.enter_context(tc.tile_pool(name="xt", bufs=6))
            nat_p = ctx.enter_context(tc.tile_pool(name="nat", bufs=4))
            exp_p = ctx.enter_context(tc.tile_pool(name="exp", bufs=3))
            rs_p = ctx.enter_context(tc.tile_pool(name="rseed", bufs=1))
            rb_p = ctx.enter_context(tc.tile_pool(name="rb", bufs=1))
            ln_p = ctx.enter_context(tc.tile_pool(name="ln", bufs=2))
            st_p = ctx.enter_context(tc.tile_pool(name="stats", bufs=4))

            # ---------- constants & weights ----------
            ident = const_p.tile([128, 128], f32)
            make_identity(nc, ident[:])
            ones_t = const_p.tile([128, 128], f32r)
            nc.vector.memset(ones_t[:].bitcast(f32), 1.0)
            eps_t = const_p.tile([128, 1], f32)
            nc.vector.memset(eps_t[:], LN_EPS)

            wq = [w_p.tile([128, HD], f32r, tag=f"wq{i}", name=f"wq{i}") for i in range(4)]
            wk = [w_p.tile([128, HD], f32r, tag=f"wk{i}", name=f"wk{i}") for i in range(4)]
            wv = [w_p.tile([128, HD], f32r, tag=f"wv{i}", name=f"wv{i}") for i in range(4)]
            for ec in range(4):
                nc.sync.dma_start(wq[ec][:], Wq_d[ec * 128 : (ec + 1) * 128, :])
                nc.sync.dma_start(wk[ec][:], Wk_d[ec * 128 : (ec + 1) * 128, :])
                nc.sync.dma_start(wv[ec][:], Wv_d[ec * 128 : (ec + 1) * 128, :])
            wf = [w_p.tile([DK, E], f32r, tag=f"wf{h}", name=f"wf{h}") for h in range(H)]
            for h in range(H):
                nc.sync.dma_start(wf[h][:], Wf_d[h * DK : (h + 1) * DK, :])
            bq_t = const_p.tile([128, PAIRS], f32)
            bk_t = const_p.tile([128, PAIRS], f32)
            bv_t = const_p.tile([DK, H], f32r)
            bf_r = const_p.tile([1, E], f32)
            ga_r = const_p.tile([1, E], f32r)
            be_r = const_p.tile([1, E], f32r)
            nc.sync.dma_start(bq_t[:], bq_d[:])
            nc.sync.dma_start(bk_t[:], bk_d[:])
            nc.sync.dma_start(bv_t[:], bv_d[:])
            nc.sync.dma_start(bf_r[:], bf_d[:])
            nc.sync.dma_start(ga_r[:], ga_d[:])
            nc.sync.dma_start(be_r[:], be_d[:])

            # bf_eff = bf + bv @ Wf  (bv folds through the final linear since
            # softmax rows sum to 1), then broadcast per-feature vectors to
            # 128 partitions via PE outer products with a ones column.
            with tc.tile_pool(name="psum_pre", bufs=2, space="PSUM") as pre_ps:
                bfe_ps = pre_ps.tile([1, E], f32, tag="bfe")
                for h in range(H):
                    nc.tensor.matmul(
                        bfe_ps[:],
                        bv_t[:, h : h + 1],
                        wf[h][:],
                        start=(h == 0),
                        stop=(h == H - 1),
                    )
                bfe_sb = const_p.tile([1, E], f32r)
                nc.vector.tensor_add(bfe_sb[:], bfe_ps[:], bf_r[:])

                bfb = act_p.tile([128, E], f32, tag="bfb")
                gab = act_p.tile([128, E], f32, tag="gab")
                beb = act_p.tile([128, E], f32, tag="beb")
                for row, dst in ((bfe_sb, bfb), (ga_r, gab), (be_r, beb)):
                    bc_ps = pre_ps.tile([128, E], f32, tag="bc")
                    nc.tensor.matmul(
                        bc_ps[:],
                        ones_t[0:1, :],
                        row[:],
                        start=True,
                        stop=True,
                    )
                    nc.vector.tensor_copy(dst[:], bc_ps[:])

            # ---------- persistent activations ----------
            qT = [act_p.tile([128, SQ], f32r, tag=f"qT{i}", name=f"qT{i}") for i in range(PAIRS)]
            kT = [act_p.tile([128, S], f32r, tag=f"kT{i}", name=f"kT{i}") for i in range(PAIRS)]
            v_aug = [act_p.tile([128, H * (DK + 1)], bf16, tag=f"vaug{i}", name=f"vaug{i}") for i in range(16)]
            zT = [act_p.tile([DK, SQ], f32r, tag=f"zT{h}", name=f"zT{h}") for h in range(H)]

            # ---------- streamed transpose + projection ----------
            def load_chunkT(src_dram, s0, xtiles):
                """DMA 512 natural rows [s0:s0+512] and PE-transpose into
                xtiles[ec][128, 512] = X.T chunk (e-major)."""
                with tc.tile_pool(name="psum_tp", bufs=4, space="PSUM") as tp_ps:
                    for st in range(4):
                        natt = nat_p.tile([128, E], f32, tag="nat")
                        nc.sync.dma_start(
                            natt[:], src_dram[s0 + st * 128 : s0 + (st + 1) * 128, :]
                        )
                        for ec in range(4):
                            tp = tp_ps.tile([128, 128], f32, tag="tp")
                            nc.tensor.transpose(
                                tp[:], natt[:, ec * 128 : (ec + 1) * 128], ident[:]
                            )
                            nc.vector.tensor_copy(
                                xtiles[ec][:, st * 128 : (st + 1) * 128], tp[:]
                            )

            with tc.tile_pool(name="psum_proj", bufs=4, space="PSUM") as proj_ps:
                # Q -> qT pairs
                for sc in range(SQ // 512):
                    qx = [xt_p.tile([128, 512], f32r, tag="xt", name=f"qx{sc}_{i}") for i in range(4)]
                    load_chunkT(Qs_d, sc * 512, qx)
                    for p in range(PAIRS):
                        pr = proj_ps.tile([128, 512], f32, tag="proj")
                        for ec in range(4):
                            nc.tensor.matmul(
                                pr[:],
                                wq[ec][:, p * 128 : (p + 1) * 128],
                                qx[ec][:],
                                start=(ec == 0),
                                stop=(ec == 3),
                            )
                        nc.vector.tensor_scalar_add(
                            qT[p][:, sc * 512 : (sc + 1) * 512], pr[:], bq_t[:, p : p + 1]
                        )
                # K -> kT pairs
                for sc in range(S // 512):
                    kx = [xt_p.tile([128, 512], f32r, tag="xt", name=f"kx{sc}_{i}") for i in range(4)]
                    load_chunkT(Kf_d, sc * 512, kx)
                    for p in range(PAIRS):
                        pr = proj_ps.tile([128, 512], f32, tag="proj")
                        for ec in range(4):
                            nc.tensor.matmul(
                                pr[:],
                                wk[ec][:, p * 128 : (p + 1) * 128],
                                kx[ec][:],
                                start=(ec == 0),
                                stop=(ec == 3),
                            )
                        nc.vector.tensor_scalar_add(
                            kT[p][:, sc * 512 : (sc + 1) * 512], pr[:], bk_t[:, p : p + 1]
                        )
                # V -> v_aug tiles [128, h*(DK+1)] with trailing ones column
                for sc in range(S // 512):
                    vx = [xt_p.tile([128, 512], f32r, tag="xt", name=f"vx{sc}_{i}") for i in range(4)]
                    load_chunkT(Vf_d, sc * 512, vx)
                    for tl in range(4):
                        tt = sc * 4 + tl
                        pr = proj_ps.tile([128, 512], f32, tag="proj")
                        for ec in range(4):
                            nc.tensor.matmul(
                                pr[:],
                                vx[ec][:, tl * 128 : (tl + 1) * 128],
                                wv[ec][:],
                                start=(ec == 0),
                                stop=(ec == 3),
                            )
                        va = v_aug[tt]
                        va3 = va[:].rearrange("p (h x) -> p h x", h=H, x=DK + 1)
                        pr3 = pr[:].rearrange("p (h d) -> p h d", h=H, d=DK)
                        nc.vector.tensor_copy(va3[:, :, 0:DK], pr3)
                        nc.vector.memset(va3[:, :, DK : DK + 1], 1.0)

            # ---------- attention (per head) ----------
            with (
                tc.tile_pool(name="psum_sc", bufs=2, space="PSUM") as sc_ps_p,
                tc.tile_pool(name="psum_pv", bufs=2, space="PSUM") as pv_ps_p,
            ):
                for h in range(H):
                    p, pb = h // 2, 64 * (h % 2)
                    pv = pv_ps_p.tile([DK + 1, SQ], f32, tag="pv")
                    for tt in range(16):
                        scs = sc_ps_p.tile([128, SQ], f32, tag="sc")
                        for qc in range(SQ // 512):
                            nc.tensor.matmul(
                                scs[:, qc * 512 : (qc + 1) * 512],
                                kT[p][pb : pb + DK, tt * 128 : (tt + 1) * 128],
                                qT[p][pb : pb + DK, qc * 512 : (qc + 1) * 512],
                                start=True,
                                stop=True,
                            )
                        ex = exp_p.tile([128, SQ], bf16, tag="exp")
                        nc.scalar.activation(
                            ex[:], scs[:], AF.Exp, scale=float(DK) ** -0.5
                        )
                        for qc in range(SQ // 512):
                            nc.tensor.matmul(
                                pv[:, qc * 512 : (qc + 1) * 512],
                                v_aug[tt][:, h * (DK + 1) : (h + 1) * (DK + 1)],
                                ex[:, qc * 512 : (qc + 1) * 512],
                                start=(tt == 0),
                                stop=(tt == 15),
                            )
                    # normalize: broadcast rowsums via PE, reciprocal, multiply
                    rseed = rs_p.tile([DK + 1, SQ], f32r, tag="rs")
                    nc.vector.tensor_copy(rseed[DK : DK + 1, :], pv[DK : DK + 1, :])
                    rb_ps = sc_ps_p.tile([DK, SQ], f32, tag="sc")
                    for qc in range(SQ // 512):
                        nc.tensor.matmul(
                            rb_ps[:, qc * 512 : (qc + 1) * 512],
                            ones_t[DK : DK + 1, 0:DK],
                            rseed[DK : DK + 1, qc * 512 : (qc + 1) * 512],
                            start=True,
                            stop=True,
                        )
                    rb_sb = rb_p.tile([DK, SQ], f32, tag="rb")
                    nc.vector.reciprocal_approx_fast(rb_sb[:], rb_ps[:])
                    nc.vector.tensor_mul(zT[h][:], pv[0:DK, :], rb_sb[:])

            # ---------- final linear + residual + LayerNorm ----------
            with tc.tile_pool(name="psum_f", bufs=2, space="PSUM") as f_ps_p:
                for qb in range(SQ // 128):
                    f_ps = f_ps_p.tile([128, E], f32, tag="f")
                    for h in range(H):
                        nc.tensor.matmul(
                            f_ps[:],
                            zT[h][:, qb * 128 : (qb + 1) * 128],
                            wf[h][:],
                            start=(h == 0),
                            stop=(h == H - 1),
                        )
                    qnat = ln_p.tile([128, E], f32, tag="qnat")
                    nc.sync.dma_start(qnat[:], Qs_d[qb * 128 : (qb + 1) * 128, :])
                    x = ln_p.tile([128, E], f32, tag="x")
                    nc.vector.tensor_add(x[:], f_ps[:], qnat[:])
                    nc.vector.tensor_add(x[:], x[:], bfb[:])
                    nm = st_p.tile([128, 1], f32, tag="nm")
                    nc.vector.tensor_reduce(
                        nm[:], x[:], mybir.AxisListType.X, mybir.AluOpType.add
                    )
                    nc.vector.tensor_scalar_mul(nm[:], nm[:], -1.0 / E)
                    sq = ln_p.tile([128, E], f32, tag="sq")
                    ss = st_p.tile([128, 1], f32, tag="ss")
                    nc.scalar.activation(
                        sq[:], x[:], AF.Square, bias=nm[:, 0:1], accum_out=ss[:]
                    )
                    sd = st_p.tile([128, 1], f32, tag="sd")
                    nc.scalar.activation(
                        sd[:], ss[:], AF.Sqrt, bias=eps_t[:, 0:1], scale=1.0 / E
                    )
                    rstd = st_p.tile([128, 1], f32, tag="rstd")
                    nc.vector.reciprocal(rstd[:], sd[:])
                    nmr = st_p.tile([128, 1], f32, tag="nmr")
                    nc.vector.tensor_mul(nmr[:], nm[:], rstd[:])
                    xn = ln_p.tile([128, E], f32, tag="xn")
                    nc.scalar.activation(
                        xn[:], x[:], AF.Identity, bias=nmr[:, 0:1], scale=rstd[:, 0:1]
                    )
                    nc.vector.tensor_mul(xn[:], xn[:], gab[:])
                    nc.vector.tensor_add(xn[:], xn[:], beb[:])
                    nc.sync.dma_start(Out_d[qb * 128 : (qb + 1) * 128, :], xn[:])

    nc.compile()
    return nc


def _get_program():
    if "nc" not in _PROGRAM_CACHE:
        _PROGRAM_CACHE["nc"] = _build_program()
    return _PROGRAM_CACHE["nc"]


def _make_in_maps(Q, K, V, Wq, bq, Wk, bk, Wv, bv, Wf, bf, gamma, beta):
    f32 = np.float32

    def per_head_w(W):  # [H, E, DK] -> [E, H*DK]
        return np.ascontiguousarray(W.transpose(1, 0, 2).reshape(E, HD), dtype=f32)

    def pair_bias(b):  # [H, DK] -> [128, PAIRS]; partition = (h%2)*64 + d
        return np.ascontiguousarray(
            b.reshape(PAIRS, 2, DK).transpose(1, 2, 0).reshape(128, PAIRS), dtype=f32
        )

    Wq_r, Wk_r, Wv_r = per_head_w(Wq), per_head_w(Wk), per_head_w(Wv)
    bq_r, bk_r = pair_bias(bq), pair_bias(bk)
    bv_r = np.ascontiguousarray(bv.reshape(H, DK).T, dtype=f32)  # [DK, H]
    Wf_c = np.ascontiguousarray(Wf, dtype=f32)
    bf_r = np.ascontiguousarray(bf.reshape(1, E), dtype=f32)
    ga_r = np.ascontiguousarray(gamma.reshape(1, E), dtype=f32)
    be_r = np.ascontiguousarray(beta.reshape(1, E), dtype=f32)

    in_maps = []
    for c in range(NCORES):
        b, qh = c // 2, c % 2
        in_maps.append(
            {
                "Qs": np.ascontiguousarray(Q[b, qh * SQ : (qh + 1) * SQ], dtype=f32),
                "Kf": np.ascontiguousarray(K[b], dtype=f32),
                "Vf": np.ascontiguousarray(V[b], dtype=f32),
                "Wq": Wq_r,
                "Wk": Wk_r,
                "Wv": Wv_r,
                "Wf": Wf_c,
                "bq_t": bq_r,
                "bk_t": bk_r,
                "bv_t": bv_r,
                "bf_r": bf_r,
                "gamma_r": ga_r,
                "beta_r": be_r,
            }
        )
    return in_maps


def run_spmd(in_maps, **kwargs):
    from concourse.bass_utils import run_bass_kernel_spmd

    nc = _get_program()
    return run_bass_kernel_spmd(nc, in_maps, list(range(NCORES)), **kwargs)


def kernel(**inputs) -> np.ndarray:
    in_maps = _make_in_maps(**inputs)
    res = run_spmd(in_maps)
    out = np.empty((B, S, E), np.float32)
    for c in range(NCORES):
        b, qh = c // 2, c % 2
        out[b, qh * SQ : (qh + 1) * SQ, :] = res.results[c]["Out"]
    return out


if __name__ == "__main__":
    rng = np.random.default_rng(0)
    print("building program...")
    _get_program()
    print("built ok")


# revision 12
# speedup vs baseline: 237.8757x; 237.8757x over previous
"""Trainium2 Bass kernel for per-head-projection MHA + residual + LayerNorm.

Problem shapes (hardcoded): B=4, S=2048, E=512, H=8, DK=64, fp32.

Sharding: 8 cores, core c -> (batch b = c//2, query-half qh = c%2).
Each core computes the full transformer block for its 1024 query rows
(using the full 2048-row K/V of its batch), so per-core outputs are
disjoint slices of the final [4, 2048, 512] output and no collectives
are needed.

Device-side layout: activations are kept transposed ([feature, seq]) so
every matmul contracts on the partition dim with zero transposes in the
attention hot loop:
  - qT/kT: [dk, seq] stored as head-pairs on 128 partitions; even head
    on partitions 0-63, odd head on 64-127, which makes the two K=64
    scores matmuls of a pair land on disjoint PE row-groups and run
    concurrently (hardware row-tiling).
  - scoresT: [keys, queries] = kT_tile.T @ qT
  - exp on ScalarE with the 1/sqrt(dk) scale folded in, bf16 out
  - PV: ctxT[dk, q] = v_aug[t, dk+1].T @ exp[t, q] in bf16; the extra
    ones column of v_aug yields the softmax denominator for free
  - final linear consumes ctxT (f32r) as the stationary operand
Matmuls run in float32r (full PE rate at N=512, ~tf32 precision); all
f32r operands are produced natively by DMA/copy so walrus accepts them.
"""

import sys

sys.path.insert(0, "/opt/trn_rl_repo")

import numpy as np

B, S, E, H, DK = 4, 2048, 512, 8, 64
NCORES = 8
SQ = (B * S) // NCORES  # 1024 query rows per core
HD = H * DK  # 512
PAIRS = H // 2
LN_EPS = 1e-5

_PROGRAM_CACHE = {}


def _build_program(repeat=1):
    from contextlib import ExitStack

    import concourse.mybir as mybir
    import concourse.tile as tile
    from concourse import bacc
    from concourse.masks import make_identity

    dt = mybir.dt
    f32, f32r, bf16 = dt.float32, dt.float32r, dt.bfloat16
    AF = mybir.ActivationFunctionType

    nc = bacc.Bacc("TRN2", target_bir_lowering=False, debug=False)

    # ---- DRAM I/O ----
    Qs_d = nc.dram_tensor("Qs", [SQ, E], f32, kind="ExternalInput").ap()
    Kf_d = nc.dram_tensor("Kf", [S, E], f32, kind="ExternalInput").ap()
    Vf_d = nc.dram_tensor("Vf", [S, E], f32, kind="ExternalInput").ap()
    Wq_d = nc.dram_tensor("Wq", [E, HD], f32r, kind="ExternalInput").ap()
    Wk_d = nc.dram_tensor("Wk", [E, HD], f32r, kind="ExternalInput").ap()
    Wv_d = nc.dram_tensor("Wv", [E, HD], f32r, kind="ExternalInput").ap()
    Wf_d = nc.dram_tensor("Wf", [HD, E], f32r, kind="ExternalInput").ap()
    bq_d = nc.dram_tensor("bq_t", [128, PAIRS], f32, kind="ExternalInput").ap()
    bk_d = nc.dram_tensor("bk_t", [128, PAIRS], f32, kind="ExternalInput").ap()
    bv_d = nc.dram_tensor("bv_t", [DK, H], f32r, kind="ExternalInput").ap()
    bf_d = nc.dram_tensor("bf_r", [1, E], f32, kind="ExternalInput").ap()
    ga_d = nc.dram_tensor("gamma_r", [1, E], f32r, kind="ExternalInput").ap()
    be_d = nc.dram_tensor("beta_r", [1, E], f32r, kind="ExternalInput").ap()
    Out_d = nc.dram_tensor("Out", [SQ, E], f32, kind="ExternalOutput").ap()

    with tile.TileContext(nc) as tc:
        for rep in range(repeat):
            _emit_body(
                nc, tc, ExitStack, mybir, make_identity, f32, f32r, bf16, AF,
                Qs_d, Kf_d, Vf_d, Wq_d, Wk_d, Wv_d, Wf_d, bq_d, bk_d, bv_d,
                bf_d, ga_d, be_d, Out_d, rep,
            )

    nc.compile()
    return nc


def _emit_body(
    nc, tc, ExitStack, mybir, make_identity, f32, f32r, bf16, AF,
    Qs_d, Kf_d, Vf_d, Wq_d, Wk_d, Wv_d, Wf_d, bq_d, bk_d, bv_d,
    bf_d, ga_d, be_d, Out_d, rep,
):
    with ExitStack() as ctx:
        const_p = ctx.enter_context(tc.tile_pool(name="const", bufs=1))
        w_p = ctx.enter_context(tc.tile_pool(name="weights", bufs=1))
        act_p = ctx.enter_context(tc.tile_pool(name="acts", bufs=1))
        xt_p = ctx.enter_context(tc.tile_pool(name="xt", bufs=6))
        nat_p = ctx.enter_context(tc.tile_pool(name="nat", bufs=4))
        exp_p = ctx.enter_context(tc.tile_pool(name="exp", bufs=3))
        rs_p = ctx.enter_context(tc.tile_pool(name="rseed", bufs=1))
        rb_p = ctx.enter_context(tc.tile_pool(name="rb", bufs=1))
        ln_p = ctx.enter_context(tc.tile_pool(name="ln", bufs=2))
        st_p = ctx.enter_context(tc.tile_pool(name="stats", bufs=4))

        # ---------- constants & weights ----------
        ident = const_p.tile([128, 128], f32)
        make_identity(nc, ident[:])
        ones_t = const_p.tile([128, 128], f32r)
        nc.vector.memset(ones_t[:].bitcast(f32), 1.0)
        eps_t = const_p.tile([128, 1], f32)
        nc.vector.memset(eps_t[:], LN_EPS)

        wq = [w_p.tile([128, HD], f32r, tag=f"wq{i}", name=f"wq{i}_{rep}") for i in range(4)]
        wk = [w_p.tile([128, HD], f32r, tag=f"wk{i}", name=f"wk{i}_{rep}") for i in range(4)]
        wv = [w_p.tile([128, HD], f32r, tag=f"wv{i}", name=f"wv{i}_{rep}") for i in range(4)]
        for ec in range(4):
            nc.sync.dma_start(wq[ec][:], Wq_d[ec * 128 : (ec + 1) * 128, :])
            nc.sync.dma_start(wk[ec][:], Wk_d[ec * 128 : (ec + 1) * 128, :])
            nc.sync.dma_start(wv[ec][:], Wv_d[ec * 128 : (ec + 1) * 128, :])
        wf = [w_p.tile([DK, E], f32r, tag=f"wf{h}", name=f"wf{h}_{rep}") for h in range(H)]
        for h in range(H):
            nc.sync.dma_start(wf[h][:], Wf_d[h * DK : (h + 1) * DK, :])
        bq_t = const_p.tile([128, PAIRS], f32)
        bk_t = const_p.tile([128, PAIRS], f32)
        bv_t = const_p.tile([DK, H], f32r)
        bf_r = const_p.tile([1, E], f32)
        ga_r = const_p.tile([1, E], f32r)
        be_r = const_p.tile([1, E], f32r)
        nc.sync.dma_start(bq_t[:], bq_d[:])
        nc.sync.dma_start(bk_t[:], bk_d[:])
        nc.sync.dma_start(bv_t[:], bv_d[:])
        nc.sync.dma_start(bf_r[:], bf_d[:])
        nc.sync.dma_start(ga_r[:], ga_d[:])
        nc.sync.dma_start(be_r[:], be_d[:])

        # bf_eff = bf + bv @ Wf (bv folds through the final linear since
        # softmax rows sum to 1); broadcast per-feature vectors to 128
        # partitions via PE outer products with a ones column.
        with tc.tile_pool(name="psum_pre", bufs=2, space="PSUM") as pre_ps:
            bfe_ps = pre_ps.tile([1, E], f32, tag="bfe")
            for h in range(H):
                nc.tensor.matmul(
                    bfe_ps[:], bv_t[:, h : h + 1], wf[h][:],
                    start=(h == 0), stop=(h == H - 1),
                )
            bfe_sb = const_p.tile([1, E], f32r)
            nc.vector.tensor_add(bfe_sb[:], bfe_ps[:], bf_r[:])

            bfb = act_p.tile([128, E], f32, tag="bfb")
            gab = act_p.tile([128, E], f32, tag="gab")
            beb = act_p.tile([128, E], f32, tag="beb")
            for row, dst in ((bfe_sb, bfb), (ga_r, gab), (be_r, beb)):
                bc_ps = pre_ps.tile([128, E], f32, tag="bc")
                nc.tensor.matmul(
                    bc_ps[:], ones_t[0:1, :], row[:], start=True, stop=True
                )
                nc.vector.tensor_copy(dst[:], bc_ps[:])

        # ---------- persistent activations ----------
        qT = [act_p.tile([128, SQ], f32r, tag=f"qT{i}", name=f"qT{i}_{rep}") for i in range(PAIRS)]
        kT = [act_p.tile([128, S], f32r, tag=f"kT{i}", name=f"kT{i}_{rep}") for i in range(PAIRS)]
        v_aug = [act_p.tile([128, H * (DK + 1)], bf16, tag=f"vaug{i}", name=f"vaug{i}_{rep}") for i in range(16)]
        zT = [act_p.tile([DK, SQ], f32r, tag=f"zT{h}", name=f"zT{h}_{rep}") for h in range(H)]

        # ---------- streamed transpose + projection ----------
        def load_chunkT(src_dram, s0, xtiles, tp_ps):
            """DMA 512 natural rows [s0:s0+512]; PE-transpose into
            xtiles[ec][128,512] = X.T chunk. Four 128x128 transposes share
            one PSUM bank and evacuate in a single ScalarE copy."""
            nats = []
            for st in range(4):
                natt = nat_p.tile([128, E], f32, tag="nat")
                nc.sync.dma_start(
                    natt[:], src_dram[s0 + st * 128 : s0 + (st + 1) * 128, :]
                )
                nats.append(natt)
            for ec in range(4):
                tp = tp_ps.tile([128, 512], f32, tag="tp")
                for st in range(4):
                    nc.tensor.transpose(
                        tp[:, st * 128 : (st + 1) * 128],
                        nats[st][:, ec * 128 : (ec + 1) * 128],
                        ident[:],
                    )
                nc.scalar.copy(xtiles[ec][:], tp[:])

        with (
            tc.tile_pool(name="psum_tp", bufs=3, space="PSUM") as tp_ps,
            tc.tile_pool(name="psum_proj", bufs=4, space="PSUM") as proj_ps,
        ):
            # Q -> qT pairs
            for sc in range(SQ // 512):
                qx = [xt_p.tile([128, 512], f32r, tag="xt", name=f"qx{sc}_{i}_{rep}") for i in range(4)]
                load_chunkT(Qs_d, sc * 512, qx, tp_ps)
                for p in range(PAIRS):
                    pr = proj_ps.tile([128, 512], f32, tag="proj")
                    for ec in range(4):
                        nc.tensor.matmul(
                            pr[:], wq[ec][:, p * 128 : (p + 1) * 128], qx[ec][:],
                            start=(ec == 0), stop=(ec == 3),
                        )
                    nc.scalar.activation(
                        qT[p][:, sc * 512 : (sc + 1) * 512], pr[:],
                        AF.Identity, bias=bq_t[:, p : p + 1],
                    )
            # K -> kT pairs
            for sc in range(S // 512):
                kx = [xt_p.tile([128, 512], f32r, tag="xt", name=f"kx{sc}_{i}_{rep}") for i in range(4)]
                load_chunkT(Kf_d, sc * 512, kx, tp_ps)
                for p in range(PAIRS):
                    pr = proj_ps.tile([128, 512], f32, tag="proj")
                    for ec in range(4):
                        nc.tensor.matmul(
                            pr[:], wk[ec][:, p * 128 : (p + 1) * 128], kx[ec][:],
                            start=(ec == 0), stop=(ec == 3),
                        )
                    nc.scalar.activation(
                        kT[p][:, sc * 512 : (sc + 1) * 512], pr[:],
                        AF.Identity, bias=bk_t[:, p : p + 1],
                    )
            # V -> v_aug tiles [128, H*(DK+1)] with trailing ones column
            for sc in range(S // 512):
                vx = [xt_p.tile([128, 512], f32r, tag="xt", name=f"vx{sc}_{i}_{rep}") for i in range(4)]
                load_chunkT(Vf_d, sc * 512, vx, tp_ps)
                for tl in range(4):
                    tt = sc * 4 + tl
                    pr = proj_ps.tile([128, 512], f32, tag="proj")
                    for ec in range(4):
                        nc.tensor.matmul(
                            pr[:], vx[ec][:, tl * 128 : (tl + 1) * 128], wv[ec][:],
                            start=(ec == 0), stop=(ec == 3),
                        )
                    va3 = v_aug[tt][:].rearrange("p (h x) -> p h x", h=H, x=DK + 1)
                    pr3 = pr[:].rearrange("p (h d) -> p h d", h=H, d=DK)
                    nc.scalar.copy(va3[:, :, 0:DK], pr3)
                    nc.vector.memset(va3[:, :, DK : DK + 1], 1.0)

        # ---------- attention: head pairs interleaved ----------
        # Even head lives on partitions 0-63, odd head on 64-127 of the
        # pair tiles, so the two K=64 scores matmuls use disjoint PE
        # row-groups and execute concurrently.
        with (
            tc.tile_pool(name="psum_sc", bufs=2, space="PSUM") as sc_ps_p,
            tc.tile_pool(name="psum_pv", bufs=2, space="PSUM") as pv_ps_p,
        ):
            for p in range(PAIRS):
                pvs = [
                    pv_ps_p.tile([DK + 1, SQ], f32, tag="pv", name=f"pv{p}_{half}_{rep}")
                    for half in range(2)
                ]
                for tt in range(16):
                    scs = [
                        sc_ps_p.tile([128, SQ], f32, tag="sc", name=f"sc{p}_{tt}_{half}_{rep}")
                        for half in range(2)
                    ]
                    for half in range(2):
                        pb = 64 * half
                        for qc in range(SQ // 512):
                            nc.tensor.matmul(
                                scs[half][:, qc * 512 : (qc + 1) * 512],
                                kT[p][pb : pb + DK, tt * 128 : (tt + 1) * 128],
                                qT[p][pb : pb + DK, qc * 512 : (qc + 1) * 512],
                                start=True, stop=True,
                            )
                    for half in range(2):
                        h = 2 * p + half
                        ex = exp_p.tile([128, SQ], bf16, tag="exp")
                        nc.scalar.activation(
                            ex[:], scs[half][:], AF.Exp, scale=float(DK) ** -0.5
                        )
                        for qc in range(SQ // 512):
                            nc.tensor.matmul(
                                pvs[half][:, qc * 512 : (qc + 1) * 512],
                                v_aug[tt][:, h * (DK + 1) : (h + 1) * (DK + 1)],
                                ex[:, qc * 512 : (qc + 1) * 512],
                                start=(tt == 0), stop=(tt == 15),
                            )
                # normalize: broadcast rowsums via PE, reciprocal, multiply
                for half in range(2):
                    h = 2 * p + half
                    pv = pvs[half]
                    rseed = rs_p.tile([DK + 1, SQ], f32r, tag="rs")
                    nc.vector.tensor_copy(rseed[DK : DK + 1, :], pv[DK : DK + 1, :])
                    rb_ps = sc_ps_p.tile([DK, SQ], f32, tag="sc")
                    for qc in range(SQ // 512):
                        nc.tensor.matmul(
                            rb_ps[:, qc * 512 : (qc + 1) * 512],
                            ones_t[DK : DK + 1, 0:DK],
                            rseed[DK : DK + 1, qc * 512 : (qc + 1) * 512],
                            start=True, stop=True,
                        )
                    rb_sb = rb_p.tile([DK, SQ], f32, tag="rb")
                    nc.vector.reciprocal_approx_fast(rb_sb[:], rb_ps[:])
                    nc.vector.tensor_mul(zT[h][:], pv[0:DK, :], rb_sb[:])

        # ---------- final linear + residual + LayerNorm ----------
        with tc.tile_pool(name="psum_f", bufs=2, space="PSUM") as f_ps_p:
            for qb in range(SQ // 128):
                f_ps = f_ps_p.tile([128, E], f32, tag="f")
                for h in range(H):
                    nc.tensor.matmul(
                        f_ps[:], zT[h][:, qb * 128 : (qb + 1) * 128], wf[h][:],
                        start=(h == 0), stop=(h == H - 1),
                    )
                qnat = ln_p.tile([128, E], f32, tag="qnat")
                nc.sync.dma_start(qnat[:], Qs_d[qb * 128 : (qb + 1) * 128, :])
                x = ln_p.tile([128, E], f32, tag="x")
                nc.vector.tensor_add(x[:], f_ps[:], qnat[:])
                nc.vector.tensor_add(x[:], x[:], bfb[:])
                nm = st_p.tile([128, 1], f32, tag="nm")
                nc.vector.tensor_reduce(
                    nm[:], x[:], mybir.AxisListType.X, mybir.AluOpType.add
                )
                nc.vector.tensor_scalar_mul(nm[:], nm[:], -1.0 / E)
                sq = ln_p.tile([128, E], f32, tag="sq")
                ss = st_p.tile([128, 1], f32, tag="ss")
                nc.scalar.activation(
                    sq[:], x[:], AF.Square, bias=nm[:, 0:1], accum_out=ss[:]
                )
                sd = st_p.tile([128, 1], f32, tag="sd")
                nc.scalar.activation(
                    sd[:], ss[:], AF.Sqrt, bias=eps_t[:, 0:1], scale=1.0 / E
                )
                rstd = st_p.tile([128, 1], f32, tag="rstd")
                nc.vector.reciprocal(rstd[:], sd[:])
                nmr = st_p.tile([128, 1], f32, tag="nmr")
                nc.vector.tensor_mul(nmr[:], nm[:], rstd[:])
                xn = ln_p.tile([128, E], f32, tag="xn")
                nc.scalar.activation(
                    xn[:], x[:], AF.Identity, bias=nmr[:, 0:1], scale=rstd[:, 0:1]
                )
                nc.vector.tensor_mul(xn[:], xn[:], gab[:])
                nc.vector.tensor_add(xn[:], xn[:], beb[:])
                nc.sync.dma_start(Out_d[qb * 128 : (qb + 1) * 128, :], xn[:])


def _get_program(repeat=1):
    key = f"nc{repeat}"
    if key not in _PROGRAM_CACHE:
        _PROGRAM_CACHE[key] = _build_program(repeat)
    return _PROGRAM_CACHE[key]


def _make_in_maps(Q, K, V, Wq, bq, Wk, bk, Wv, bv, Wf, bf, gamma, beta):
    f32 = np.float32

    def per_head_w(W):  # [H, E, DK] -> [E, H*DK]
        return np.ascontiguousarray(W.transpose(1, 0, 2).reshape(E, HD), dtype=f32)

    def pair_bias(b):  # [H, DK] -> [128, PAIRS]; partition = (h%2)*64 + d
        return np.ascontiguousarray(
            b.reshape(PAIRS, 2, DK).transpose(1, 2, 0).reshape(128, PAIRS), dtype=f32
        )

    Wq_r, Wk_r, Wv_r = per_head_w(Wq), per_head_w(Wk), per_head_w(Wv)
    bq_r, bk_r = pair_bias(bq), pair_bias(bk)
    bv_r = np.ascontiguousarray(bv.reshape(H, DK).T, dtype=f32)  # [DK, H]
    Wf_c = np.ascontiguousarray(Wf, dtype=f32)
    bf_r = np.ascontiguousarray(bf.reshape(1, E), dtype=f32)
    ga_r = np.ascontiguousarray(gamma.reshape(1, E), dtype=f32)
    be_r = np.ascontiguousarray(beta.reshape(1, E), dtype=f32)

    in_maps = []
    for c in range(NCORES):
        b, qh = c // 2, c % 2
        in_maps.append(
            {
                "Qs": np.ascontiguousarray(Q[b, qh * SQ : (qh + 1) * SQ], dtype=f32),
                "Kf": np.ascontiguousarray(K[b], dtype=f32),
                "Vf": np.ascontiguousarray(V[b], dtype=f32),
                "Wq": Wq_r,
                "Wk": Wk_r,
                "Wv": Wv_r,
                "Wf": Wf_c,
                "bq_t": bq_r,
                "bk_t": bk_r,
                "bv_t": bv_r,
                "bf_r": bf_r,
                "gamma_r": ga_r,
                "beta_r": be_r,
            }
        )
    return in_maps


def run_spmd(in_maps, **kwargs):
    from concourse.bass_utils import run_bass_kernel_spmd

    nc = _get_program()
    return run_bass_kernel_spmd(nc, in_maps, list(range(NCORES)), **kwargs)


def kernel(**inputs) -> np.ndarray:
    in_maps = _make_in_maps(**inputs)
    res = run_spmd(in_maps)
    out = np.empty((B, S, E), np.float32)
    for c in range(NCORES):
        b, qh = c // 2, c % 2
        out[b, qh * SQ : (qh + 1) * SQ, :] = res.results[c]["Out"]
    return out


if __name__ == "__main__":
    import time

    t0 = time.time()
    _get_program()
    print(f"built ok in {time.time() - t0:.1f}s")
